# revision 2
# baseline (speedup 1.0000x reference)
"""FFT-based DCT-II on 8 trn2 NeuronCores (rev F).

Per core (256 rows): Makhoul DCT->real-FFT, four-step radix-64x64, twiddles
folded into stage-2 tables, conjugate symmetry (66 stage-1 slots incl. two
zero columns), mid-transpose via DRAM roundtrip. fp16 operands, fp32 psum,
fp16 output (cast in copyback; host converts to fp32).

vs rev E: x1 split across sync+scalar queues (stage 1 starts ~2.5us in);
hh on the gpsimd SWDGE queue; T2 read in fine 4-m chunks on 3 queues so
stage 2 starts right after the last T write; y accumulated in SBUF fp16
and written in four large DMAs.
"""

import numpy as np

N = 4096
R = 2048
RPC = 256

_state = {}


def _tables():
    n1 = np.arange(64)[:, None].astype(np.float64)
    j = np.arange(33)[None, :].astype(np.float64)
    F1c = np.cos(2 * np.pi * n1 * j / 64)
    F1s = -np.sin(2 * np.pi * n1 * j / 64)
    F1 = np.concatenate([F1c, F1s], axis=1)  # [64, 66]; cols 33 & 65 are 0
    f1_np = np.vstack([F1, F1]).astype(np.float16)  # [128, 66]

    n2v = np.arange(64)[:, None].astype(np.float64)
    k2v = np.arange(64)[None, :].astype(np.float64)

    def HH_single(k1):
        k = 64 * k2v + k1
        Gc = np.cos(2 * np.pi * n2v * k / N)
        Gs = -np.sin(2 * np.pi * n2v * k / N)
        cosE = np.cos(np.pi * k / (2 * N))
        sinE = np.sin(np.pi * k / (2 * N))
        sigma = 1.0 if k1 <= 32 else -1.0
        H1 = cosE * Gc + sinE * Gs
        H2 = sigma * (sinE * Gc - cosE * Gs)
        return np.concatenate([H1, H2], axis=0)  # [128, 64]

    HH = np.zeros((33, 128, 128))
    for a in range(1, 32):
        HH[a][:, :64] = HH_single(a)
        HH[a][:, 64:] = HH_single(64 - a)
    HH[0][:, :64] = HH_single(0)
    HH[32][:, 64:] = HH_single(32)
    # t2 partitions come from the (n c) DMA merge: p = 2*n2 + c
    rowperm = np.empty(128, dtype=np.int64)
    for n2 in range(64):
        for c in range(2):
            rowperm[2 * n2 + c] = c * 64 + n2
    HH = HH[:, rowperm, :]
    hh_np = HH.transpose(1, 0, 2).astype(np.float16).copy()  # [128, 33, 128]

    k1_arr = np.empty(64, dtype=np.int64)
    for a in range(32):
        k1_arr[2 * a] = a
        k1_arr[2 * a + 1] = (64 - a) if a > 0 else 32
    return f1_np, hh_np, k1_arr


# m-slot schedule for stage 2: 9 read chunks; chunk 0 carries m={0,32}
# (slot a=0 accumulates both), then chunks of 4.
_M_CHUNKS = [[0, 32]] + [list(range(1 + 4 * i, 1 + 4 * i + 4)) for i in range(8)]
# slot a -> (chunk index, index within chunk)
_M_POS = {}
for _ci, _ch in enumerate(_M_CHUNKS):
    for _mi, _m in enumerate(_ch):
        _M_POS[_m] = (_ci, _mi)


def _build():
    import concourse.tile as tile
    from concourse import bacc, mybir

    f16 = mybir.dt.float16
    f32 = mybir.dt.float32

    nc = bacc.Bacc("TRN2", target_bir_lowering=False, debug=False, num_devices=8)
    x1_d = nc.dram_tensor("x1", [128, 8192], f16, kind="ExternalInput").ap()
    f1_d = nc.dram_tensor("f1", [128, 66], f16, kind="ExternalInput").ap()
    hh_d = nc.dram_tensor("hh", [128, 33, 128], f16, kind="ExternalInput").ap()
    y_d = nc.dram_tensor("y", [128, 32, 256], f16, kind="ExternalOutput").ap()

    with tile.TileContext(nc) as tc:
        with (
            tc.tile_pool(name="const", bufs=1) as const,
            tc.tile_pool(name="data", bufs=1) as data,
            tc.tile_pool(name="dram", bufs=1, space="DRAM") as dram,
            tc.tile_pool(name="ps1", bufs=3, space="PSUM") as ps1,
            tc.tile_pool(name="ps2", bufs=2, space="PSUM") as ps2,
        ):
            f1_sb = const.tile([128, 66], f16)
            hh_sb = const.tile([128, 33, 128], f16)
            nc.sync.dma_start(f1_sb[:], f1_d)

            # x1 in 4 chunks split across the two HWDGE queues so the
            # first chunks land early; hh rides the gpsimd SWDGE queue.
            x1_g = []
            for g in range(4):
                xg = data.tile([128, 2048], f16, name=f"x1_{g}")
                eng = nc.sync if g % 2 == 0 else nc.scalar
                eng.dma_start(xg[:], x1_d[:, 2048 * g : 2048 * g + 2048])
                x1_g.append(xg)
            nc.gpsimd.dma_start(hh_sb[:], hh_d)

            # T in DRAM [n2, c, m, r]: per-p writes are (c m)-major with
            # 512B r-runs; reads merge (n c) into 128 partitions.
            t_dram = dram.tile([64, 2, 33, 256], f16)  # [n2, c, m, r]
            t_sb = data.tile([66, 64, 256], f16)  # [s=(c,m), n2, r]

            # stage 1: p-major, h-alternating emission (adjacent MMs hit
            # different PE row groups and overlap in the array).
            cb = 0
            for p in range(8):
                tiles = [
                    ps1.tile([66, 2, 512], f32, name=f"s1ps_{p}_{h}", tag="s1ps")
                    for h in range(2)
                ]
                for j in range(2):
                    for h in range(2):
                        f = 2 * p + j
                        g, sl = f // 4, (f % 4) * 512
                        nc.tensor.matmul(
                            tiles[h][:, j, :],
                            f1_sb[64 * h : 64 * h + 64, :],
                            x1_g[g][64 * h : 64 * h + 64, sl : sl + 512],
                            start=True,
                            stop=True,
                        )
                for h in range(2):
                    dst = t_sb[:, 8 * p : 8 * p + 8, 128 * h : 128 * h + 128]
                    src = tiles[h][:].rearrange("s j (a b) -> s (j a) b", a=4)
                    if cb % 2 == 0:
                        nc.vector.tensor_copy(dst, src)
                    else:
                        nc.scalar.copy(dst, src)
                    cb += 1
                # write this p's 8-n2 slice (full r, 512B runs)
                eng = nc.sync if p % 2 == 0 else nc.scalar
                eng.dma_start(
                    t_dram[8 * p : 8 * p + 8].rearrange("n c m r -> (c m) n r"),
                    t_sb[:, 8 * p : 8 * p + 8, :],
                )

            # T2 read in fine m-chunks over 3 queues; stage 2 per chunk.
            t_rd = t_dram[:].rearrange("n c m r -> (n c) m r")
            t2_tiles = []
            rd_engines = [nc.sync, nc.scalar, nc.gpsimd]
            for ci, ch in enumerate(_M_CHUNKS):
                t2 = data.tile([128, len(ch), 256], f16, name=f"t2_{ci}")
                eng = rd_engines[ci % 3]
                if ch == [0, 32]:
                    eng.dma_start(t2[:, 0:1, :], t_rd[:, 0:1, :])
                    eng.dma_start(t2[:, 1:2, :], t_rd[:, 32:33, :])
                else:
                    eng.dma_start(t2[:], t_rd[:, ch[0] : ch[0] + len(ch), :])
                t2_tiles.append(t2)

            def t2_slice(m):
                ci, mi = _M_POS[m]
                return t2_tiles[ci][:, mi, :]

            # stage 2: 16 psum tiles, each two a's; a=0 accumulates m=0 and
            # m=32. Output cast to fp16 into a big staging tile, written in
            # 4 large DMAs.
            y_sb = data.tile([128, 32, 256], f16)
            for q in range(16):
                ps = ps2.tile([128, 512], f32)
                for i in range(2):
                    a = 2 * q + i
                    out = ps[:, 256 * i : 256 * i + 256]
                    if a == 0:
                        nc.tensor.matmul(
                            out, hh_sb[:, 0, :], t2_slice(0),
                            start=True, stop=False,
                        )
                        nc.tensor.matmul(
                            out, hh_sb[:, 32, :], t2_slice(32),
                            start=False, stop=True,
                        )
                    else:
                        nc.tensor.matmul(
                            out, hh_sb[:, a, :], t2_slice(a),
                            start=True, stop=True,
                        )
                dst = y_sb[:, 2 * q : 2 * q + 2, :]
                src = ps[:].rearrange("p (a r) -> p a r", a=2)
                if q % 2 == 0:
                    nc.vector.tensor_copy(dst, src)
                else:
                    nc.scalar.copy(dst, src)
                # write out every 4 slots (8 a's) as they complete
                if q % 4 == 3:
                    q0 = q - 3
                    eng = nc.sync if (q // 4) % 2 == 0 else nc.scalar
                    eng.dma_start(
                        y_d[:, 2 * q0 : 2 * q0 + 8, :],
                        y_sb[:, 2 * q0 : 2 * q0 + 8, :],
                    )

    nc.compile()
    return nc


def _pack_x1(x_rows):
    v = np.empty_like(x_rows)
    v[:, : N // 2] = x_rows[:, 0::2]
    v[:, N // 2 :] = x_rows[:, 1::2][:, ::-1]
    x1 = v.reshape(2, 128, 64, 64).transpose(0, 2, 3, 1).reshape(128, 8192)
    return np.ascontiguousarray(x1.astype(np.float16))


def kernel(x, _trace: bool = False):
    from concourse.bass_utils import run_bass_kernel_spmd

    x = np.asarray(x, dtype=np.float32)
    assert x.shape == (R, N)
    if "nc" not in _state:
        _state["nc"] = _build()
        _state["tables"] = _tables()
    nc = _state["nc"]
    f1_np, hh_np, k1_arr = _state["tables"]

    in_maps = []
    for c in range(8):
        in_maps.append(
            {
                "x1": _pack_x1(x[c * RPC : (c + 1) * RPC]),
                "f1": f1_np,
                "hh": hh_np,
            }
        )

    res = run_bass_kernel_spmd(nc, in_maps, list(range(8)), trace=_trace)

    y = np.empty((R, N), dtype=np.float32)
    for c in range(8):
        ydev = res.results[c]["y"].astype(np.float32)  # [128, 32, 256] fp16
        # partitions = (d, k2); slot index (a, d) -> k1 = k1_arr[2a+d]
        perm = ydev.reshape(2, 64, 32, 256).transpose(3, 1, 2, 0).reshape(RPC, 64, 64)
        yc = np.empty((RPC, 64, 64), dtype=np.float32)
        yc[:, :, k1_arr] = perm
        y[c * RPC : (c + 1) * RPC] = yc.reshape(RPC, N)
    if _trace:
        _state["last_result"] = res
    return y
